# revision 5
# baseline (speedup 1.0000x reference)
"""Distributed GQA attention (B=2,T=2048,C=2048,H=16,KV=4,D=128, RoPE, causal)
for one TRN2 chip (8 NeuronCores).

Sharding (no collectives except KV AllGather): core c -> batch b=c//4,
stripe s=c%4. Each core handles query rows {r : r % 4 == s} of its batch
(512 rows, interleaved so causal spans are shape-uniform across cores ->
one SPMD graph), computes K/V for its 512-token chunk (KV proj sharded,
AllGather within the 4-core batch group), and produces complete output
rows. Host reassembles by stripe.

Per-core pipeline (tensor-engine-dense by construction):
  KVproj chunk (bf16)  -> K^T[d,t] (+RoPE), V[t,d]; staged to DRAM and
    AllGather'd on the sync DMA queue while Qproj runs.
  Qproj (bf16, weights streamed on the scalar DMA queue) -> Q^T (+RoPE/sqrt(D))
  attention, scores transposed: S^T[k,(h4,q)] = K-tile^T . Q(4 heads)
    + staircase causal mask (DVE/Pool split so exp never waits on the
    mask engine), exp on ACT -> P^T bf16; the softmax denominator is a
    bf16 running sum of P^T tiles on DVE + ONE ones-vector matmul per
    (qt,g) (instead of one per k-tile); staircase tiles use partial-free
    APs (q >= 32*ktl) in scores/mask/exp/AV.
  Oproj interleaved per q-tile into the attention loop (chain per output
    c-block spliced after each (qt,g) group) so the tensor engine stays
    busy while ACT paces the exp chain. Output DMA'd in bf16.
"""

import numpy as np
import ml_dtypes

import concourse.bass as bass
import concourse.tile as tile
from concourse import bacc, mybir
from concourse.bass_utils import run_bass_kernel_spmd

B, T, C = 2, 2048, 2048
H, KV, D = 16, 4, 128
G4 = H // KV            # q heads per kv head
THETA = 10000.0
P = 128
CT = C // P             # 16 c-tiles
TQ = 512                # queries per core
NQT = TQ // P           # 4 q-tiles
NTT = T // P            # 16 token tiles
NCC = C // 512          # 4 output column blocks
MASK_VAL = -1e5

f32 = mybir.dt.float32
bf16 = mybir.dt.bfloat16

_compiled = {}


def _build():
    nc = bacc.Bacc("TRN2", target_bir_lowering=False, debug=False, num_devices=8)
    xq_e = nc.dram_tensor("xq", [P, CT * TQ], bf16, kind="ExternalInput")
    xkv_e = nc.dram_tensor("xkv", [P, CT * TQ], bf16, kind="ExternalInput")  # chunk, pre-tiled [p,(ct t)]
    wq_e = nc.dram_tensor("wq", [P, H * CT * D], bf16, kind="ExternalInput")  # [p,(h ct d)]
    wkv_e = nc.dram_tensor("wkv", [P, CT * 2 * KV * D], bf16, kind="ExternalInput")  # [p,(ct n)]
    wo_e = nc.dram_tensor("wo", [P, NCC * H * 512], bf16, kind="ExternalInput")  # [p,(cc hh c)]
    cq_e = nc.dram_tensor("cos_q", [D, TQ], f32, kind="ExternalInput")
    sq_e = nc.dram_tensor("sin_q", [D, TQ], f32, kind="ExternalInput")
    ck_e = nc.dram_tensor("cos_k", [D, TQ], bf16, kind="ExternalInput")  # chunk positions
    sk_e = nc.dram_tensor("sin_k", [D, TQ], bf16, kind="ExternalInput")
    mk_e = nc.dram_tensor("mask", [P, NQT * P], bf16, kind="ExternalInput")
    out_e = nc.dram_tensor("out", [TQ, C], bf16, kind="ExternalOutput")

    NR = 4  # ranks per batch group

    from contextlib import ExitStack

    with tile.TileContext(nc) as tc, ExitStack() as top:
        persist = top.enter_context(tc.tile_pool(name="persist", bufs=1))

        mask_t = persist.tile([P, NQT, P], bf16)
        ones_col = persist.tile([P, 1], bf16)
        nc.vector.memset(ones_col[:], 1.0)
        qhat = persist.tile([D, H, TQ], bf16)
        khat = persist.tile([D, NR, KV, TQ], bf16)
        vsb = persist.tile([P, NTT, KV * D], bf16)
        yhat = persist.tile([D, H, TQ], bf16)

        # Q operand pools (DMAs on the scalar queue so the sync queue stays
        # clear for the collective staging + readback).
        qstack = ExitStack()
        xqp = qstack.enter_context(tc.tile_pool(name="xqp", bufs=1))
        wstream = qstack.enter_context(tc.tile_pool(name="wqstream", bufs=4))
        tabq = qstack.enter_context(tc.tile_pool(name="tabq", bufs=1))

        # ---- KV chunk projection (bf16) + AllGather ---------------------
        with tc.tile_pool(name="kvchunk", bufs=1) as kvc, \
             tc.tile_pool(name="dram", bufs=1, space="DRAM") as dram, \
             tc.tile_pool(name="ps_kvp", bufs=3, space="PSUM") as ps_kv, \
             tc.tile_pool(name="ropek", bufs=2) as ropekp:
            # per-c-tile tiles so the first KV matmuls start as soon as the
            # first chunks land (exact per-tile deps).
            xkv_r = xkv_e.ap().rearrange("p (ct t) -> p ct t", ct=CT)
            wkv_r = wkv_e.ap().rearrange("p (ct n) -> p ct n", ct=CT)
            xkvs, wks = [], []
            for ct in range(CT):
                xt = kvc.tile([P, TQ], bf16, name=f"xkv{ct}")
                nc.sync.dma_start(xt[:], xkv_r[:, ct])
                xkvs.append(xt)
                wt = kvc.tile([P, KV * D], bf16, name=f"wk{ct}")
                nc.sync.dma_start(wt[:], wkv_r[:, ct, 0:KV * D])
                wks.append(wt)
            cos_k = kvc.tile([D, TQ], bf16)
            nc.sync.dma_start(cos_k[:], ck_e.ap())
            sin_k = kvc.tile([D, TQ], bf16)
            nc.sync.dma_start(sin_k[:], sk_e.ap())

            kchunk = kvc.tile([D, KV, TQ], bf16)
            vchunk = kvc.tile([P, NQT, KV * D], bf16)

            for g in range(KV):
                ps = ps_kv.tile([P, TQ], f32, tag="ps_kv")
                for ct in range(CT):
                    nc.tensor.matmul(ps[:], wks[ct][:, g * D:(g + 1) * D],
                                     xkvs[ct][:],
                                     start=(ct == 0), stop=(ct == CT - 1))
                tmp = ropekp.tile([D, TQ], f32, tag="rope_k")
                nc.vector.tensor_copy(tmp[0:64, :], ps[64:128, :])
                nc.vector.tensor_copy(tmp[64:128, :], ps[0:64, :])
                ksl = kchunk[:, g, :]
                nc.vector.tensor_mul(ksl, ps[:], cos_k[:])
                nc.vector.tensor_mul(tmp[:], tmp[:], sin_k[:])
                nc.vector.tensor_add(ksl, ksl, tmp[:])

            # V weights on the sync queue (after the K-side tiles).
            wkvv = kvc.tile([P, CT, KV * D], bf16)
            nc.sync.dma_start(wkvv[:], wkv_r[:, :, KV * D:2 * KV * D])

            # Q-side operands on the scalar queue (parallel to the above).
            xq = xqp.tile([P, CT, TQ], bf16)
            nc.scalar.dma_start(xq[:], xq_e.ap().rearrange("p (ct q) -> p ct q", ct=CT))
            cos_q = tabq.tile([D, TQ], f32)
            nc.scalar.dma_start(cos_q[:], cq_e.ap())
            sin_q = tabq.tile([D, TQ], f32)
            nc.scalar.dma_start(sin_q[:], sq_e.ap())

            for ttl in range(NQT):
                ps = ps_kv.tile([P, KV * D], f32, tag="ps_kv")
                for ct in range(CT):
                    nc.tensor.matmul(ps[:], xkvs[ct][:, ttl * P:(ttl + 1) * P],
                                     wkvv[:, ct, :],
                                     start=(ct == 0), stop=(ct == CT - 1))
                nc.vector.tensor_copy(vchunk[:, ttl, :], ps[:])

            cc_in = dram.tile([2, P, KV, TQ], bf16)
            cc_out = dram.tile([NR, 2, P, KV, TQ], bf16)
            nc.sync.dma_start(cc_in[0], kchunk[:])
            nc.sync.dma_start(cc_in[1], vchunk[:].rearrange("p t n -> p (t n)").rearrange("p (g x) -> p g x", g=KV))
            nc.gpsimd.collective_compute(
                "AllGather",
                mybir.AluOpType.bypass,
                replica_groups=[[0, 1, 2, 3], [4, 5, 6, 7]],
                ins=[cc_in[:].opt()],
                outs=[cc_out[:].opt()],
            )
            # khat[d, r, g, t] <- cc_out[r, 0, d, g, t]: contiguous 2KB runs
            nc.sync.dma_start(
                khat[:], cc_out[:, 0].rearrange("r d g t -> d r g t"))
            # vsb[p, (r ttl), n] <- cc_out[r, 1, p, ttl, n]
            nc.sync.dma_start(
                vsb[:].rearrange("p (r ttl) n -> p r ttl n", r=NR),
                cc_out[:, 1].rearrange("r p g x -> p r (g x)").rearrange("p r (ttl n) -> p r ttl n", ttl=NQT))
            nc.sync.dma_start(mask_t[:], mk_e.ap().rearrange("p (kt q) -> p kt q", kt=NQT))

        # ---- Q projection (bf16, weights streamed via scalar queue) -----
        with tc.tile_pool(name="ps_qp", bufs=3, space="PSUM") as ps_q, \
             tc.tile_pool(name="ropeq", bufs=2) as ropep:
            for h in range(H):
                wqt = wstream.tile([P, CT, D], bf16, tag="wq")
                nc.scalar.dma_start(
                    wqt[:], wq_e.ap().rearrange("p (h ct d) -> p h ct d", h=H, ct=CT)[:, h])
                ps = ps_q.tile([P, TQ], f32, tag="ps_q")
                for ct in range(CT):
                    nc.tensor.matmul(ps[:], wqt[:, ct, :], xq[:, ct, :],
                                     start=(ct == 0), stop=(ct == CT - 1))
                tmp = ropep.tile([D, TQ], f32, tag="rope_q")
                nc.vector.tensor_copy(tmp[0:64, :], ps[64:128, :])
                nc.vector.tensor_copy(tmp[64:128, :], ps[0:64, :])
                qsl = qhat[:, h, :]
                nc.vector.tensor_mul(qsl, ps[:], cos_q[:])
                nc.vector.tensor_mul(tmp[:], tmp[:], sin_q[:])
                nc.vector.tensor_add(qsl, qsl, tmp[:])
        qstack.close()

        # ---- attention (scores transposed) + interleaved Oproj ----------
        with tc.tile_pool(name="wop", bufs=1) as wop, \
             tc.tile_pool(name="ptile", bufs=8) as ptp, \
             tc.tile_pool(name="ptil", bufs=2) as ptilp, \
             tc.tile_pool(name="small", bufs=3) as small, \
             tc.tile_pool(name="outp", bufs=2) as outp, \
             tc.tile_pool(name="ps_s", bufs=4, space="PSUM") as ps_sp, \
             tc.tile_pool(name="ps_y", bufs=2, space="PSUM") as ps_yp, \
             tc.tile_pool(name="ps_den", bufs=1, space="PSUM") as ps_denp, \
             tc.tile_pool(name="ps_o", bufs=1, space="PSUM") as ps_op:
            # Wo resident, streamed per c-block on the scalar queue; the
            # dispatches sit after the last Wq load on that queue, so the
            # transfers land just as the first Oproj chains need them.
            wo_r = wo_e.ap().rearrange("p (cc hh c) -> p cc hh c", cc=NCC, hh=H)
            wos = []
            for cc in range(NCC):
                wt = wop.tile([P, H, 512], bf16, name=f"wo{cc}")
                nc.scalar.dma_start(wt[:], wo_r[:, cc])
                wos.append(wt)

            def oproj_chain(qt, cc):
                ps_o = ps_op.tile([P, 512], f32, tag="ps_o")
                for hh in range(H):
                    nc.tensor.matmul(ps_o[:], yhat[:, hh, qt * P:(qt + 1) * P],
                                     wos[cc][:, hh, :],
                                     start=(hh == 0), stop=(hh == H - 1))
                osb = outp.tile([P, 512], bf16, tag="osb")
                nc.vector.tensor_copy(osb[:], ps_o[:])
                nc.sync.dma_start(
                    out_e.ap()[qt * P:(qt + 1) * P, cc * 512:(cc + 1) * 512], osb[:])

            for qt in range(NQT):
                nkt = 4 * (qt + 1)
                for g in range(KV):
                    # free layout is (q, h4) so staircase partial slices
                    # (q >= q0) are contiguous prefixes -> 2-dim APs.
                    ps_y = ps_yp.tile([P, P, G4], f32, tag="ps_y")
                    ptil = ptilp.tile([P, P, G4], bf16, tag="ptil")
                    qh_qh = qhat[:, g * G4:(g + 1) * G4, qt * P:(qt + 1) * P] \
                        .rearrange("d h q -> d q h")

                    def emit_scores(kt):
                        ktl = kt - (nkt - 4)  # staircase index when >= 0
                        q0 = 32 * ktl if ktl > 0 else 0
                        ps_s = ps_sp.tile([P, P, G4], f32, tag="ps_s")
                        nc.tensor.matmul(
                            ps_s[:, q0:, :],
                            khat[:, kt // 4, g, (kt % 4) * P:(kt % 4 + 1) * P],
                            qh_qh[:, q0:, :],
                            start=True, stop=True)
                        if ktl >= 0:
                            nc.vector.tensor_add(
                                ps_s[:, q0:, :], ps_s[:, q0:, :],
                                mask_t[:, ktl, q0:, None].to_broadcast((P, P - q0, G4)))
                        pt = ptp.tile([P, P, G4], bf16, tag="pt")
                        nc.scalar.activation(pt[:, q0:, :], ps_s[:, q0:, :],
                                             mybir.ActivationFunctionType.Exp)
                        return (kt, q0, pt)

                    def emit_av(kt, q0, pt):
                        nc.tensor.matmul(ps_y[:, q0:, :],
                                         vsb[:, kt, g * D:(g + 1) * D],
                                         pt[:, q0:, :],
                                         start=(kt == 0), stop=(kt == nkt - 1))
                        # P-tile running sum (softmax denominator), emitted
                        # trailing the exp by the AV lookahead so DVE's
                        # in-order stream never blocks a mask add on an exp.
                        if kt == 0:
                            nc.vector.tensor_copy(ptil[:], pt[:])
                        else:
                            nc.vector.tensor_add(ptil[:, q0:, :], ptil[:, q0:, :],
                                                 pt[:, q0:, :])

                    pend = []
                    for kt in range(nkt):
                        pend.append(emit_scores(kt))
                        if len(pend) > 3:
                            emit_av(*pend.pop(0))
                    for item in pend:
                        emit_av(*item)

                    ps_den = ps_denp.tile([1, P * G4], f32, tag="ps_den")
                    nc.tensor.matmul(ps_den[:], ones_col[:],
                                     ptil[:].rearrange("p q h -> p (q h)"),
                                     start=True, stop=True)
                    den = small.tile([1, P * G4], f32, tag="den")
                    nc.vector.tensor_copy(den[:], ps_den[:])
                    rec = small.tile([1, P * G4], f32, tag="rec")
                    nc.vector.reciprocal_approx_fast(rec[:], den[:])
                    bc = small.tile([P, P, G4], f32, tag="bc")
                    nc.gpsimd.partition_broadcast(bc[:], rec[:])
                    ysl = yhat[:, g * G4:(g + 1) * G4, qt * P:(qt + 1) * P] \
                        .rearrange("d h q -> d q h")
                    nc.vector.tensor_mul(ysl, ps_y[:], bc[:])

                    # splice previous q-tile's Oproj chain to keep the PE
                    # busy while ACT works through the exp chain.
                    if qt > 0:
                        oproj_chain(qt - 1, g)
            for cc in range(NCC):
                oproj_chain(NQT - 1, cc)

    nc.compile()
    return nc


def _rope_tables():
    freqs = 1.0 / (THETA ** (np.arange(0, D, 2, dtype=np.float64) / D))
    ang = np.arange(T, dtype=np.float64)[:, None] * freqs[None, :]
    emb = np.concatenate([ang, ang], axis=-1)          # [T, D]
    return np.cos(emb), np.sin(emb)                    # [T, D] each


def _prep_inputs(x, Wq, Wkv, Wo):
    cos, sin = _rope_tables()
    sgn = np.where(np.arange(D) < D // 2, -1.0, 1.0)   # sign for shifted term
    inv = 1.0 / np.sqrt(D)
    cosT = np.ascontiguousarray(cos.T)                 # [D, T]
    sinTs = np.ascontiguousarray(sin.T) * sgn[:, None]

    # pre-tiled layouts: every DMA reads contiguous per-partition runs
    # wq [p, (h ct d)]: wq[p, h, ct, d] = Wq.T[ct*128+p, h*128+d]
    wq_t = np.ascontiguousarray(
        Wq.T.reshape(16, 128, 16, 128).transpose(1, 2, 0, 3).reshape(128, -1)
    ).astype(ml_dtypes.bfloat16)
    # wkv [p, (ct n)]: wkv[p, ct, n] = Wkv.T[ct*128+p, n]
    wkv_t = np.ascontiguousarray(
        Wkv.T.reshape(16, 128, 1024).transpose(1, 0, 2).reshape(128, -1)
    ).astype(ml_dtypes.bfloat16)
    # wo [p, (cc hh c)]: wo[p, cc, hh, c] = Wo.T[hh*128+p, cc*512+c]
    wo_t = np.ascontiguousarray(
        Wo.T.reshape(16, 128, 4, 512).transpose(1, 2, 0, 3).reshape(128, -1)
    ).astype(ml_dtypes.bfloat16)

    in_maps = []
    for c in range(8):
        b, s = c // 4, c % 4
        rows = np.arange(s, T, 4)
        xq = np.ascontiguousarray(
            x[b][rows, :].T.reshape(16, 128, 512).transpose(1, 0, 2).reshape(128, -1)
        ).astype(ml_dtypes.bfloat16)  # [p, (ct q)]
        ch = np.arange(512 * s, 512 * (s + 1))
        xkv = np.ascontiguousarray(
            x[b][ch, :].T.reshape(16, 128, 512).transpose(1, 0, 2).reshape(128, -1)
        ).astype(ml_dtypes.bfloat16)  # [p, (ct t)] chunk
        cq = np.ascontiguousarray(cosT[:, rows] * inv, dtype=np.float32)
        sq = np.ascontiguousarray(sinTs[:, rows] * inv, dtype=np.float32)
        # staircase mask, transposed: [k-window j, q i]; visible iff j <= 4i+s
        j = np.arange(TQ)[:, None]
        i = np.arange(P)[None, :]
        mask = np.where(j <= 4 * i + s, 0.0, MASK_VAL).astype(np.float32)
        # pre-tiled [p, (kt q)]: mask_t[p, kt, q] = mask[kt*128+p, q]
        mask = np.ascontiguousarray(
            mask.reshape(4, 128, 128).transpose(1, 0, 2).reshape(128, -1)
        ).astype(ml_dtypes.bfloat16)
        in_maps.append({
            "xq": xq, "xkv": xkv,
            "wq": wq_t, "wkv": wkv_t, "wo": wo_t,
            "cos_q": cq, "sin_q": sq,
            "cos_k": np.ascontiguousarray(cosT[:, ch]).astype(ml_dtypes.bfloat16),
            "sin_k": np.ascontiguousarray(sinTs[:, ch]).astype(ml_dtypes.bfloat16),
            "mask": mask,
        })
    return in_maps


def _unshard(results):
    full = np.empty((B, T, C), dtype=np.float32)
    for c in range(8):
        b, s = c // 4, c % 4
        full[b, s::4, :] = results[c]["out"].astype(np.float32)
    return full


def run(x, Wq, Wkv, Wo, trace=False, trace_kwargs=None):
    import time
    if "nc" not in _compiled:
        _compiled["nc"] = _build()
    nc = _compiled["nc"]
    in_maps = _prep_inputs(np.asarray(x), np.asarray(Wq), np.asarray(Wkv), np.asarray(Wo))
    last_err = None
    for attempt in range(3):
        try:
            res = run_bass_kernel_spmd(nc, in_maps, core_ids=list(range(8)), trace=trace,
                                       **(trace_kwargs or {}))
            return _unshard(res.results), res
        except Exception as e:  # transient NRT device errors recover on retry
            last_err = e
            time.sleep(5)
    raise last_err


def kernel(x, Wq, Wkv, Wo):
    out, _ = run(x, Wq, Wkv, Wo, trace=False)
    return out


# revision 12
# speedup vs baseline: 1.1333x; 1.1333x over previous
"""Distributed GQA attention (B=2,T=2048,C=2048,H=16,KV=4,D=128, RoPE, causal)
for one TRN2 chip (8 NeuronCores).

Sharding (no collectives except KV AllGather): core c -> batch b=c//4,
stripe s=c%4. Each core handles query rows {r : r % 4 == s} of its batch
(512 rows, interleaved so causal spans are shape-uniform across cores ->
one SPMD graph), computes K/V for its 512-token chunk (KV proj sharded,
AllGather within the 4-core batch group), and produces complete output
rows. Host reassembles by stripe.

Per-core pipeline (tensor-engine-dense by construction):
  KVproj chunk (bf16)  -> K^T[d,t] (+RoPE), V[t,d]; staged to DRAM and
    AllGather'd on the sync DMA queue while Qproj runs.
  Qproj (bf16, weights streamed on the scalar DMA queue) -> Q^T (+RoPE/sqrt(D))
  attention, scores transposed: S^T[k,(h4,q)] = K-tile^T . Q(4 heads)
    + staircase causal mask (DVE/Pool split so exp never waits on the
    mask engine), exp on ACT -> P^T bf16; the softmax denominator is a
    bf16 running sum of P^T tiles on DVE + ONE ones-vector matmul per
    (qt,g) (instead of one per k-tile); staircase tiles use partial-free
    APs (q >= 32*ktl) in scores/mask/exp/AV.
  Oproj interleaved per q-tile into the attention loop (chain per output
    c-block spliced after each (qt,g) group) so the tensor engine stays
    busy while ACT paces the exp chain. Output DMA'd in bf16.
"""

import numpy as np
import ml_dtypes

import concourse.bass as bass
import concourse.tile as tile
from concourse import bacc, mybir
from concourse.bass_utils import run_bass_kernel_spmd

B, T, C = 2, 2048, 2048
H, KV, D = 16, 4, 128
G4 = H // KV            # q heads per kv head
THETA = 10000.0
P = 128
CT = C // P             # 16 c-tiles
TQ = 512                # queries per core
NQT = TQ // P           # 4 q-tiles
NTT = T // P            # 16 token tiles
NCC = C // 512          # 4 output column blocks
MASK_VAL = -1e5

f32 = mybir.dt.float32
bf16 = mybir.dt.bfloat16

_compiled = {}


def _build():
    nc = bacc.Bacc("TRN2", target_bir_lowering=False, debug=False, num_devices=8)
    xq_e = nc.dram_tensor("xq", [P, CT * TQ], bf16, kind="ExternalInput")
    xkv_e = nc.dram_tensor("xkv", [P, CT * TQ], bf16, kind="ExternalInput")  # chunk, pre-tiled [p,(ct t)]
    wq_e = nc.dram_tensor("wq", [P, H * CT * D], bf16, kind="ExternalInput")  # [p,(h ct d)]
    wkv_e = nc.dram_tensor("wkv", [P, CT * 2 * KV * D], bf16, kind="ExternalInput")  # [p,(ct n)]
    wo_e = nc.dram_tensor("wo", [P, NCC * H * 512], bf16, kind="ExternalInput")  # [p,(cc hh c)]
    cq_e = nc.dram_tensor("cos_q", [D, TQ], f32, kind="ExternalInput")
    sq_e = nc.dram_tensor("sin_q", [D, TQ], f32, kind="ExternalInput")
    ck_e = nc.dram_tensor("cos_k", [D, TQ], bf16, kind="ExternalInput")  # chunk positions
    sk_e = nc.dram_tensor("sin_k", [D, TQ], bf16, kind="ExternalInput")
    mk_e = nc.dram_tensor("mask", [P, NQT * P], bf16, kind="ExternalInput")
    out_e = nc.dram_tensor("out", [TQ, C], bf16, kind="ExternalOutput")

    NR = 4  # ranks per batch group

    from contextlib import ExitStack

    with tile.TileContext(nc) as tc, ExitStack() as top:
        persist = top.enter_context(tc.tile_pool(name="persist", bufs=1))

        mask_t = persist.tile([P, NQT, P], bf16)
        ones_col = persist.tile([P, 1], bf16)
        nc.vector.memset(ones_col[:], 1.0)
        # qhat laid out [d, g, q, j] so the scores moving operand (q-major,
        # head-minor) is contiguous.
        qhat = persist.tile([D, KV, TQ, G4], bf16)
        khat = persist.tile([D, NR, KV, TQ], bf16)
        vsb = persist.tile([P, NTT, KV * D], bf16)
        yhat = persist.tile([D, H, TQ], bf16)

        # Q operand pools (DMAs on the scalar queue so the sync queue stays
        # clear for the collective staging + readback).
        qstack = ExitStack()
        xqp = qstack.enter_context(tc.tile_pool(name="xqp", bufs=1))
        wstream = qstack.enter_context(tc.tile_pool(name="wqstream", bufs=2))
        tabq = qstack.enter_context(tc.tile_pool(name="tabq", bufs=1))

        # ---- KV chunk projection (bf16) + AllGather ---------------------
        with tc.tile_pool(name="kvchunk", bufs=1) as kvc, \
             tc.tile_pool(name="dram", bufs=1, space="DRAM") as dram, \
             tc.tile_pool(name="ps_kvp", bufs=1, space="PSUM") as ps_kv, \
             tc.tile_pool(name="ropek", bufs=2) as ropekp:
            # per-c-tile tiles so the first KV matmuls start as soon as the
            # first chunks land (exact per-tile deps). ct-outer loop with all
            # 8 K/V accumulators resident (8 PSUM banks) -> the tensor engine
            # starts ~1us in and streams DMA-paced, instead of waiting for
            # the full x-chunk before the first chain.
            xkv_r = xkv_e.ap().rearrange("p (ct t) -> p ct t", ct=CT)
            wkv_r = wkv_e.ap().rearrange("p (ct n) -> p ct n", ct=CT)
            xkvs, wkvs = [], []
            for ct in range(CT):
                xt = kvc.tile([P, TQ], bf16, name=f"xkv{ct}")
                nc.sync.dma_start(xt[:], xkv_r[:, ct])
                xkvs.append(xt)
                wt = kvc.tile([P, 2 * KV * D], bf16, name=f"wkv{ct}")
                nc.sync.dma_start(wt[:], wkv_r[:, ct])
                wkvs.append(wt)
            cos_k = kvc.tile([D, TQ], bf16)
            nc.sync.dma_start(cos_k[:], ck_e.ap())
            sin_k = kvc.tile([D, TQ], bf16)
            nc.sync.dma_start(sin_k[:], sk_e.ap())

            kchunk = kvc.tile([D, KV, TQ], bf16)
            vchunk = kvc.tile([P, NQT, KV * D], bf16)

            # Q-side operands on the scalar queue (parallel to the above).
            xq = xqp.tile([P, CT, TQ], bf16)
            nc.scalar.dma_start(xq[:], xq_e.ap().rearrange("p (ct q) -> p ct q", ct=CT))
            cos_q = tabq.tile([D, TQ], f32)
            nc.scalar.dma_start(cos_q[:], cq_e.ap())
            sin_q = tabq.tile([D, TQ], f32)
            nc.scalar.dma_start(sin_q[:], sq_e.ap())

            ps_ks = [ps_kv.tile([P, TQ], f32, name=f"ps_k{g}") for g in range(KV)]
            ps_vs = [ps_kv.tile([P, KV * D], f32, name=f"ps_v{t}") for t in range(NQT)]
            for ct in range(CT):
                for g in range(KV):
                    nc.tensor.matmul(ps_ks[g][:], wkvs[ct][:, g * D:(g + 1) * D],
                                     xkvs[ct][:],
                                     start=(ct == 0), stop=(ct == CT - 1))
                for ttl in range(NQT):
                    nc.tensor.matmul(ps_vs[ttl][:], xkvs[ct][:, ttl * P:(ttl + 1) * P],
                                     wkvs[ct][:, KV * D:],
                                     start=(ct == 0), stop=(ct == CT - 1))

            # RoPE on DVE; V drains on ACT in parallel.
            for g in range(KV):
                ps = ps_ks[g]
                tmp = ropekp.tile([D, TQ], f32, tag="rope_k")
                nc.vector.tensor_copy(tmp[0:64, :], ps[64:128, :])
                nc.vector.tensor_copy(tmp[64:128, :], ps[0:64, :])
                ksl = kchunk[:, g, :]
                nc.vector.tensor_mul(ksl, ps[:], cos_k[:])
                nc.vector.tensor_mul(tmp[:], tmp[:], sin_k[:])
                nc.vector.tensor_add(ksl, ksl, tmp[:])
            for ttl in range(NQT):
                nc.scalar.activation(vchunk[:, ttl, :], ps_vs[ttl][:],
                                     mybir.ActivationFunctionType.Copy)

            cc_in = dram.tile([2, P, KV, TQ], bf16)
            cc_out = dram.tile([NR, 2, P, KV, TQ], bf16)
            nc.sync.dma_start(cc_in[0], kchunk[:])
            nc.sync.dma_start(cc_in[1], vchunk[:].rearrange("p t n -> p (t n)").rearrange("p (g x) -> p g x", g=KV))
            nc.gpsimd.collective_compute(
                "AllGather",
                mybir.AluOpType.bypass,
                replica_groups=[[0, 1, 2, 3], [4, 5, 6, 7]],
                ins=[cc_in[:].opt()],
                outs=[cc_out[:].opt()],
            )
            # khat[d, r, g, t] <- cc_out[r, 0, d, g, t]: contiguous 2KB runs
            nc.sync.dma_start(
                khat[:], cc_out[:, 0].rearrange("r d g t -> d r g t"))
            # vsb[p, (r ttl), n] <- cc_out[r, 1, p, ttl, n]
            nc.sync.dma_start(
                vsb[:].rearrange("p (r ttl) n -> p r ttl n", r=NR),
                cc_out[:, 1].rearrange("r p g x -> p r (g x)").rearrange("p r (ttl n) -> p r ttl n", ttl=NQT))
            nc.sync.dma_start(mask_t[:], mk_e.ap().rearrange("p (kt q) -> p kt q", kt=NQT))

        # ---- Q projection (bf16, weights streamed via scalar queue) -----
        with tc.tile_pool(name="ps_qp", bufs=3, space="PSUM") as ps_q, \
             tc.tile_pool(name="ropeq", bufs=2) as ropep:
            for h in range(H):
                wqt = wstream.tile([P, CT, D], bf16, tag="wq")
                nc.scalar.dma_start(
                    wqt[:], wq_e.ap().rearrange("p (h ct d) -> p h ct d", h=H, ct=CT)[:, h])
                ps = ps_q.tile([P, TQ], f32, tag="ps_q")
                for ct in range(CT):
                    nc.tensor.matmul(ps[:], wqt[:, ct, :], xq[:, ct, :],
                                     start=(ct == 0), stop=(ct == CT - 1))
                tmp = ropep.tile([D, TQ], f32, tag="rope_q")
                nc.vector.tensor_copy(tmp[0:64, :], ps[64:128, :])
                nc.vector.tensor_copy(tmp[64:128, :], ps[0:64, :])
                qsl = qhat[:, h // G4, :, h % G4]
                nc.vector.tensor_mul(qsl, ps[:], cos_q[:])
                nc.vector.tensor_mul(tmp[:], tmp[:], sin_q[:])
                nc.vector.tensor_add(qsl, qsl, tmp[:])
        qstack.close()

        # ---- attention (scores transposed) + interleaved Oproj ----------
        with tc.tile_pool(name="wop", bufs=1) as wop, \
             tc.tile_pool(name="ptile", bufs=8) as ptp, \
             tc.tile_pool(name="ptil", bufs=2) as ptilp, \
             tc.tile_pool(name="small", bufs=3) as small, \
             tc.tile_pool(name="outp", bufs=2) as outp, \
             tc.tile_pool(name="ps_s", bufs=3, space="PSUM") as ps_sp, \
             tc.tile_pool(name="ps_y", bufs=2, space="PSUM") as ps_yp, \
             tc.tile_pool(name="ps_den", bufs=1, space="PSUM") as ps_denp, \
             tc.tile_pool(name="ps_o", bufs=2, space="PSUM") as ps_op:
            # Wo resident, streamed per c-block on the scalar queue; the
            # dispatches sit after the last Wq load on that queue, so the
            # transfers land just as the first Oproj chains need them.
            wo_r = wo_e.ap().rearrange("p (cc hh c) -> p cc hh c", cc=NCC, hh=H)
            wos = []
            for cc in range(NCC):
                wt = wop.tile([P, H, 512], bf16, name=f"wo{cc}")
                nc.scalar.dma_start(wt[:], wo_r[:, cc])
                wos.append(wt)

            # Oproj chains are emitted as generators and "pumped" one matmul
            # at a time between attention matmuls: the PE gets filler work
            # whenever the exp chain would otherwise let it idle (keeping the
            # p-state up), and the Oproj phase all but disappears.
            from collections import deque
            pending = deque()

            def oproj_gen(qt, cc):
                ps_o = ps_op.tile([P, 512], f32, tag="ps_o")
                for hh in range(H):
                    nc.tensor.matmul(ps_o[:], yhat[:, hh, qt * P:(qt + 1) * P],
                                     wos[cc][:, hh, :],
                                     start=(hh == 0), stop=(hh == H - 1))
                    yield
                osb = outp.tile([P, 512], bf16, tag="osb")
                nc.scalar.activation(osb[:], ps_o[:],
                                     mybir.ActivationFunctionType.Copy)
                nc.sync.dma_start(
                    out_e.ap()[qt * P:(qt + 1) * P, cc * 512:(cc + 1) * 512], osb[:])

            def pump(n):
                for _ in range(n):
                    if not pending:
                        return
                    try:
                        next(pending[0])
                    except StopIteration:
                        pending.popleft()

            for qt in range(NQT):
                nkt = 4 * (qt + 1)
                if qt > 0:
                    for cc in range(NCC):
                        pending.append(oproj_gen(qt - 1, cc))
                for g in range(KV):
                    # free layout is (q, h4) so staircase partial slices
                    # (q >= q0) are contiguous prefixes -> 2-dim APs.
                    ps_y = ps_yp.tile([P, P, G4], f32, tag="ps_y")
                    ptil = ptilp.tile([P, P, G4], bf16, tag="ptil")

                    def emit_scores(kt):
                        ktl = kt - (nkt - 4)  # staircase index when >= 0
                        q0 = 32 * ktl if ktl > 0 else 0
                        ps_s = ps_sp.tile([P, P, G4], f32, tag="ps_s")
                        nc.tensor.matmul(
                            ps_s[:, q0:, :],
                            khat[:, kt // 4, g, (kt % 4) * P:(kt % 4 + 1) * P],
                            qhat[:, g, qt * P + q0:(qt + 1) * P, :],
                            start=True, stop=True)
                        if ktl >= 0:
                            nc.vector.tensor_add(
                                ps_s[:, q0:, :], ps_s[:, q0:, :],
                                mask_t[:, ktl, q0:, None].to_broadcast((P, P - q0, G4)))
                        pt = ptp.tile([P, P, G4], bf16, tag="pt")
                        if q0:
                            # zero the never-exp'd prefix so the full-tile
                            # (fast-path contiguous) denominator add is exact
                            nc.vector.memset(pt[:, :q0, :], 0.0)
                        nc.scalar.activation(pt[:, q0:, :], ps_s[:, q0:, :],
                                             mybir.ActivationFunctionType.Exp)
                        return (kt, q0, pt)

                    def emit_av(kt, q0, pt):
                        nc.tensor.matmul(ps_y[:, q0:, :],
                                         vsb[:, kt, g * D:(g + 1) * D],
                                         pt[:, q0:, :],
                                         start=(kt == 0), stop=(kt == nkt - 1))
                        pump(2)
                        # P-tile running sum (softmax denominator), emitted
                        # trailing the exp by the AV lookahead so DVE's
                        # in-order stream never blocks a mask add on an exp.
                        if kt == 0:
                            nc.vector.tensor_copy(ptil[:], pt[:])
                        else:
                            nc.vector.tensor_add(ptil[:], ptil[:], pt[:])

                    pend = []
                    for kt in range(nkt):
                        pend.append(emit_scores(kt))
                        if len(pend) > 2:
                            emit_av(*pend.pop(0))
                    for item in pend:
                        emit_av(*item)

                    ps_den = ps_denp.tile([1, P * G4], f32, tag="ps_den")
                    nc.tensor.matmul(ps_den[:], ones_col[:],
                                     ptil[:].rearrange("p q h -> p (q h)"),
                                     start=True, stop=True)
                    den = small.tile([1, P * G4], f32, tag="den")
                    nc.scalar.activation(den[:], ps_den[:],
                                         mybir.ActivationFunctionType.Copy)
                    rec = small.tile([1, P * G4], f32, tag="rec")
                    nc.vector.reciprocal_approx_fast(rec[:], den[:])
                    bc = small.tile([P, P, G4], f32, tag="bc")
                    nc.gpsimd.partition_broadcast(bc[:], rec[:])
                    ysl = yhat[:, g * G4:(g + 1) * G4, qt * P:(qt + 1) * P] \
                        .rearrange("d h q -> d q h")
                    nc.vector.tensor_mul(ysl, ps_y[:], bc[:])
                    pump(4)
            for cc in range(NCC):
                pending.append(oproj_gen(NQT - 1, cc))
            pump(10 ** 6)

    nc.compile()
    return nc


def _rope_tables():
    freqs = 1.0 / (THETA ** (np.arange(0, D, 2, dtype=np.float64) / D))
    ang = np.arange(T, dtype=np.float64)[:, None] * freqs[None, :]
    emb = np.concatenate([ang, ang], axis=-1)          # [T, D]
    return np.cos(emb), np.sin(emb)                    # [T, D] each


def _prep_inputs(x, Wq, Wkv, Wo):
    cos, sin = _rope_tables()
    sgn = np.where(np.arange(D) < D // 2, -1.0, 1.0)   # sign for shifted term
    inv = 1.0 / np.sqrt(D)
    cosT = np.ascontiguousarray(cos.T)                 # [D, T]
    sinTs = np.ascontiguousarray(sin.T) * sgn[:, None]

    # pre-tiled layouts: every DMA reads contiguous per-partition runs
    # wq [p, (h ct d)]: wq[p, h, ct, d] = Wq.T[ct*128+p, h*128+d]
    wq_t = np.ascontiguousarray(
        Wq.T.reshape(16, 128, 16, 128).transpose(1, 2, 0, 3).reshape(128, -1)
    ).astype(ml_dtypes.bfloat16)
    # wkv [p, (ct n)]: wkv[p, ct, n] = Wkv.T[ct*128+p, n]
    wkv_t = np.ascontiguousarray(
        Wkv.T.reshape(16, 128, 1024).transpose(1, 0, 2).reshape(128, -1)
    ).astype(ml_dtypes.bfloat16)
    # wo [p, (cc hh c)]: wo[p, cc, hh, c] = Wo.T[hh*128+p, cc*512+c]
    wo_t = np.ascontiguousarray(
        Wo.T.reshape(16, 128, 4, 512).transpose(1, 2, 0, 3).reshape(128, -1)
    ).astype(ml_dtypes.bfloat16)

    in_maps = []
    for c in range(8):
        b, s = c // 4, c % 4
        rows = np.arange(s, T, 4)
        xq = np.ascontiguousarray(
            x[b][rows, :].T.reshape(16, 128, 512).transpose(1, 0, 2).reshape(128, -1)
        ).astype(ml_dtypes.bfloat16)  # [p, (ct q)]
        ch = np.arange(512 * s, 512 * (s + 1))
        xkv = np.ascontiguousarray(
            x[b][ch, :].T.reshape(16, 128, 512).transpose(1, 0, 2).reshape(128, -1)
        ).astype(ml_dtypes.bfloat16)  # [p, (ct t)] chunk
        cq = np.ascontiguousarray(cosT[:, rows] * inv, dtype=np.float32)
        sq = np.ascontiguousarray(sinTs[:, rows] * inv, dtype=np.float32)
        # staircase mask, transposed: [k-window j, q i]; visible iff j <= 4i+s
        j = np.arange(TQ)[:, None]
        i = np.arange(P)[None, :]
        mask = np.where(j <= 4 * i + s, 0.0, MASK_VAL).astype(np.float32)
        # pre-tiled [p, (kt q)]: mask_t[p, kt, q] = mask[kt*128+p, q]
        mask = np.ascontiguousarray(
            mask.reshape(4, 128, 128).transpose(1, 0, 2).reshape(128, -1)
        ).astype(ml_dtypes.bfloat16)
        in_maps.append({
            "xq": xq, "xkv": xkv,
            "wq": wq_t, "wkv": wkv_t, "wo": wo_t,
            "cos_q": cq, "sin_q": sq,
            "cos_k": np.ascontiguousarray(cosT[:, ch]).astype(ml_dtypes.bfloat16),
            "sin_k": np.ascontiguousarray(sinTs[:, ch]).astype(ml_dtypes.bfloat16),
            "mask": mask,
        })
    return in_maps


def _unshard(results):
    full = np.empty((B, T, C), dtype=np.float32)
    for c in range(8):
        b, s = c // 4, c % 4
        full[b, s::4, :] = results[c]["out"].astype(np.float32)
    return full


def run(x, Wq, Wkv, Wo, trace=False, trace_kwargs=None):
    import time
    if "nc" not in _compiled:
        _compiled["nc"] = _build()
    nc = _compiled["nc"]
    in_maps = _prep_inputs(np.asarray(x), np.asarray(Wq), np.asarray(Wkv), np.asarray(Wo))
    last_err = None
    for attempt in range(3):
        try:
            res = run_bass_kernel_spmd(nc, in_maps, core_ids=list(range(8)), trace=trace,
                                       **(trace_kwargs or {}))
            return _unshard(res.results), res
        except Exception as e:  # transient NRT device errors recover on retry
            last_err = e
            time.sleep(5)
    raise last_err


def kernel(x, Wq, Wkv, Wo):
    out, _ = run(x, Wq, Wkv, Wo, trace=False)
    return out
